# revision 2
# baseline (speedup 1.0000x reference)
"""Trainium2 Bass kernel for nn_DepthAwareGATv2 (2-layer GATv2 GNN, 8 cores).

Sharding: core k owns node rows [k*6250, (k+1)*6250); edges sharded by the
dst owner and sorted by dst. Uniform tiling: tile t of every core covers
local nodes [124t, 124t+124) with a global edge capacity CAP (multiple of
128); padding edges use slot 127 (outside the 124-wide one-hot) and src 0.

Per-layer node tables are computed shard-wise, all-gathered, then per-edge
rows are fetched by indirect DMA keyed on src. dst-side values come from
contiguous local rows, broadcast to edges through one-hot matmuls on PE; the
segment softmax is aggregated as num/den (no max subtraction) in PSUM.
"""

import math
import os

import numpy as np

NCORES = 8
LAST_EXEC_NS = None
LAST_SCOPES = None
P = 128
NEG_SLOPE = 0.2
NPB = 124            # nodes per tile

_CACHE = {}


def _bf16_np(a):
    import ml_dtypes
    return np.ascontiguousarray(np.asarray(a, dtype=np.float32).astype(ml_dtypes.bfloat16))


def _preprocess(edge_index, n, nsh, nshp):
    src_all = edge_index[0].astype(np.int64)
    dst_all = edge_index[1].astype(np.int64)
    t_cnt = (nsh + NPB - 1) // NPB
    percore = []
    cap = 0
    for k in range(NCORES):
        n0 = k * nsh
        m = (dst_all // nsh) == k
        src, dst = src_all[m], dst_all[m]
        order = np.argsort(dst, kind="stable")
        src, dst = src[order], dst[order]
        blk = (dst - n0) // NPB
        cnt = np.bincount(blk, minlength=t_cnt)
        cap = max(cap, int(cnt.max()))
        percore.append((n0, src, dst, cnt))
    cap = int(math.ceil(cap / 128) * 128)
    ch = cap // P
    out = []
    for n0, src, dst, cnt in percore:
        src_idx = np.zeros((t_cnt, cap), np.int32)
        slot_idx = np.full((t_cnt, cap), 127, np.int32)
        starts = np.zeros(t_cnt + 1, np.int64)
        np.cumsum(cnt, out=starts[1:])
        for t in range(t_cnt):
            e0, e1 = starts[t], starts[t + 1]
            ne = e1 - e0
            # src ids remapped into the all-gathered (stride nshp) row space
            s = src[e0:e1]
            src_idx[t, :ne] = (s // nsh) * nshp + (s % nsh)
            slot_idx[t, :ne] = dst[e0:e1] - n0 - t * NPB
        si = src_idx.reshape(t_cnt, ch, P).transpose(2, 0, 1).reshape(P, t_cnt * ch)
        sl = slot_idx.reshape(t_cnt, ch, P).transpose(2, 0, 1).reshape(P, t_cnt * ch)
        out.append((np.ascontiguousarray(si),
                    np.ascontiguousarray(sl.astype(np.float32))))
    return cap, out


def _fold_weights(I):
    H, HID = I["c1_att_src"].shape
    F = I["x"].shape[1]

    def v(lin_w, att):
        return (lin_w.reshape(lin_w.shape[0], H, HID) * att[None]).sum(-1)

    W = {}
    V1s, V1d = v(I["c1_lin_w"], I["c1_att_src"]), v(I["c1_lin_w"], I["c1_att_dst"])
    V2s, V2d = v(I["c2_lin_w"], I["c2_att_src"]), v(I["c2_lin_w"], I["c2_att_dst"])
    G1 = np.einsum("jhc,hc->jh", I["c1_edge_w"].reshape(H, H, HID), I["c1_att_edge"])
    G2 = np.einsum("jhc,hc->jh", I["c2_edge_w"].reshape(H, H, HID), I["c2_att_edge"])
    W["mboth"] = np.concatenate([I["ee_w2"] @ G1, I["ee_w2"] @ G2], axis=1)  # [16,16]
    dd = np.concatenate([I["ee_b2"] @ G1, I["ee_b2"] @ G2])                  # [16]
    W["ddc"] = np.tile(dd[None, :], (P, 1)).astype(np.float32)
    W["w1e"] = (I["ee_w1"] * (1.0 / 128.0) ** np.arange(1, 5)[:, None])      # [4,16]
    W["eeb1"] = I["ee_b1"].reshape(16, 1).astype(np.float32)
    W["wnode1"] = np.concatenate(
        [np.eye(F, dtype=np.float32), I["c1_lin_w"], V1s, V1d, I["in_w"]], axis=1)
    W["wnode2"] = np.concatenate([I["c2_lin_w"], V2s, V2d], axis=1)
    W["jkw"] = np.stack([I["jk_w"][0:128], I["jk_w"][128:256], I["jk_w"][256:384]],
                        axis=1)                                              # [128,3,128]
    W["clsw"] = I["cls_w"]                                                   # [128,40]
    # fold jk_b into the classifier bias; replicate biases across partitions
    clsb = I["cls_b"] + I["jk_b"] @ I["cls_w"]
    W["clsb"] = np.tile(clsb[None, :], (P, 1)).astype(np.float32)            # [128,40]
    W["inb"] = np.tile(I["in_b"][None, :], (P, 1)).astype(np.float32)
    W["c1b"] = np.tile(I["c1_bias"][None, :], (P, 1)).astype(np.float32)
    W["c2b"] = np.tile(I["c2_bias"][None, :], (P, 1)).astype(np.float32)
    W["ln1g"] = np.tile(I["n1_g"][None, :], (P, 1)).astype(np.float32)
    W["ln1b"] = np.tile(I["n1_b"][None, :], (P, 1)).astype(np.float32)
    W["ln2g"] = np.tile(I["n2_g"][None, :], (P, 1)).astype(np.float32)
    W["ln2b"] = np.tile(I["n2_b"][None, :], (P, 1)).astype(np.float32)
    return W


def _build(cfg):
    import concourse.bass as bass
    import concourse.mybir as mybir
    from concourse.bacc import Bacc
    from concourse.tile import TileContext
    from concourse.masks import make_identity

    f32 = mybir.dt.float32
    bf = mybir.dt.bfloat16
    i32 = mybir.dt.int32
    AF = mybir.ActivationFunctionType
    OP = mybir.AluOpType
    AX = mybir.AxisListType

    N, NSH, NSHP, CAP, ncores = (cfg["N"], cfg["NSH"], cfg["NSHP"], cfg["CAP"],
                                 cfg["ncores"])
    T = (NSH + NPB - 1) // NPB
    CH = CAP // P
    NCH = NSHP // P
    NG = NSHP * ncores       # gathered table rows

    nc = Bacc(num_devices=ncores)

    xt = nc.dram_tensor("xt", [P, NSHP], bf, kind="ExternalInput")
    src_idx = nc.dram_tensor("src_idx", [P, T * CH], i32, kind="ExternalInput")
    slot_f = nc.dram_tensor("slot_f", [P, T * CH], f32, kind="ExternalInput")
    iota_c = nc.dram_tensor("iota_c", [P, P], f32, kind="ExternalInput")
    wnode1 = nc.dram_tensor("wnode1", [P, 400], bf, kind="ExternalInput")
    wnode2 = nc.dram_tensor("wnode2", [P, 144], bf, kind="ExternalInput")
    w1e = nc.dram_tensor("w1e", [4, 16], bf, kind="ExternalInput")
    mboth = nc.dram_tensor("mboth", [16, 16], bf, kind="ExternalInput")
    eeb1 = nc.dram_tensor("eeb1", [16, 1], f32, kind="ExternalInput")
    ddc = nc.dram_tensor("ddc", [P, 16], f32, kind="ExternalInput")
    jkw = nc.dram_tensor("jkw", [P, 3, 128], bf, kind="ExternalInput")
    clsw = nc.dram_tensor("clsw", [P, 40], bf, kind="ExternalInput")
    clsb = nc.dram_tensor("clsb", [P, 40], f32, kind="ExternalInput")
    biases = nc.dram_tensor("biases", [P, 8, 128], f32, kind="ExternalInput")
    # biases idx: 0 inb, 1 c1b, 2 c2b, 3 ln1g, 4 ln1b, 5 ln2g, 6 ln2b, 7 pad

    out_t = nc.dram_tensor("out", [NSHP, 40], f32, kind="ExternalOutput")

    with TileContext(nc) as tc:
        with (
            tc.tile_pool(name="dram", bufs=1, space="DRAM") as dpool,
            tc.tile_pool(name="consts", bufs=1) as cpool,
            tc.tile_pool(name="gather", bufs=2) as gpool,
            tc.tile_pool(name="work", bufs=2) as wpool,
            tc.tile_pool(name="small", bufs=3) as spool,
            tc.tile_pool(name="ohpool", bufs=2 * CH + 2) as ohpool,
            tc.tile_pool(name="pstr", bufs=2, space="PSUM") as pstr,
            tc.tile_pool(name="psadst", bufs=2, space="PSUM") as psadst,
            tc.tile_pool(name="psagg", bufs=2, space="PSUM") as psagg,
            tc.tile_pool(name="pshh", bufs=1, space="PSUM") as pshh,
            tc.tile_pool(name="npool", bufs=2) as npool,
        ):
            shared = {"addr_space": "Shared"} if ncores > 1 else {}
            t1_loc = dpool.tile([NSHP, 272], bf, name="t1_loc")
            t1_full = dpool.tile([NG, 272], bf, name="t1_full", **shared)
            t2_loc = dpool.tile([NSHP, 144], bf, name="t2_loc")
            t2_full = dpool.tile([NG, 144], bf, name="t2_full", **shared)
            h0_loc = dpool.tile([NSHP, 128], f32, name="h0_loc")
            h1_loc = dpool.tile([NSHP, 128], f32, name="h1_loc")
            agg1 = dpool.tile([NSHP, 136], f32, name="agg1")
            agg2 = dpool.tile([NSHP, 136], f32, name="agg2")
            ae_dram = dpool.tile([T, P, CH * 16], bf, name="ae_dram")

            def ld(pool, shape, dt_, src):
                t = pool.tile(shape, dt_, name=f"c_{src.name}")
                nc.sync.dma_start(out=t[:], in_=src[:])
                return t

            iota_t = ld(cpool, [P, P], f32, iota_c)
            wn1_t = ld(cpool, [P, 400], bf, wnode1)
            wn2_t = ld(cpool, [P, 144], bf, wnode2)
            w1e_t = ld(cpool, [4, 16], bf, w1e)
            mb_t = ld(cpool, [16, 16], bf, mboth)
            eeb1_t = ld(cpool, [16, 1], f32, eeb1)
            ddc_t = ld(cpool, [P, 16], f32, ddc)
            jkw_t = ld(cpool, [P, 3, 128], bf, jkw)
            clsw_t = ld(cpool, [P, 40], bf, clsw)
            clsb_t = ld(cpool, [P, 40], f32, clsb)
            bias_t = ld(cpool, [P, 8, 128], f32, biases)
            ident = cpool.tile([P, P], bf, name="ident")
            make_identity(nc, ident[:])
            eps_t = cpool.tile([P, 1], f32, name="eps_t")
            nc.vector.memset(eps_t[:], 1e-5)
            sc128_t = cpool.tile([P, 1], f32, name="sc128_t")
            nc.vector.memset(sc128_t[:], 1.0 / 128)

            # ================= P1: layer-1 node tables + h0 =================
            with nc.named_scope("p1"):
                def p1_body(i):
                    xchunk = npool.tile([P, P], bf, tag="xchunk")
                    nc.sync.dma_start(out=xchunk[:], in_=xt[:, bass.ds(i * P, P)])
                    ps = pstr.tile([P, 400], f32, tag="tr")
                    nc.tensor.matmul(out=ps[:], lhsT=xchunk[:],
                                     rhs=wn1_t[:], start=True, stop=True)
                    t1row = npool.tile([P, 272], bf, tag="t1row")
                    nc.vector.tensor_copy(out=t1row[:], in_=ps[:, 0:272])
                    nc.sync.dma_start(out=t1_loc[bass.ds(i * P, P), :], in_=t1row[:])
                    h0row = npool.tile([P, 128], f32, tag="h0row")
                    nc.vector.tensor_copy(out=h0row[:], in_=ps[:, 272:400])
                    nc.sync.dma_start(out=h0_loc[bass.ds(i * P, P), :], in_=h0row[:])
                tc.For_i_unrolled(0, NCH, 1, p1_body, max_unroll=4)

            with nc.named_scope("ag1"):
                if ncores > 1:
                    nc.gpsimd.collective_compute(
                        "AllGather", mybir.AluOpType.bypass,
                        replica_groups=[list(range(ncores))],
                        ins=[t1_loc[:]], outs=[t1_full[:]],
                    )
                else:
                    nc.sync.dma_start(out=t1_full[:], in_=t1_loc[:])

            # ================= edge/layer passes =================
            def layer_pass(tbl, tbl_loc, tcols, agg_dst, layer, scope):
                as_off = tcols - 16
                ad_off = tcols - 8
                xs_off = 128 if layer == 1 else 0

                def body(i):
                    st_idx = spool.tile([P, CH], i32, tag=f"sti{layer}")
                    nc.sync.dma_start(out=st_idx[:],
                                      in_=src_idx[:, bass.ds(i * CH, CH)])
                    st_slot = spool.tile([P, CH], f32, tag=f"sts{layer}")
                    nc.sync.dma_start(out=st_slot[:],
                                      in_=slot_f[:, bass.ds(i * CH, CH)])
                    g = gpool.tile([P, CH, tcols], bf, tag=f"g{layer}")
                    for c in range(CH):
                        nc.gpsimd.indirect_dma_start(
                            out=g[:, c, :], out_offset=None, in_=tbl[:],
                            in_offset=bass.IndirectOffsetOnAxis(
                                ap=st_idx[:, c:c + 1], axis=0),
                        )
                    dslots = spool.tile([NPB, tcols], bf, tag=f"ds{layer}")
                    nc.sync.dma_start(out=dslots[:],
                                      in_=tbl_loc[bass.ds(i * NPB, NPB), :])
                    adst_ps = psadst.tile([P, CH, 8], f32, tag="adst")
                    aggp = psagg.tile([NPB, 136], f32, tag="agg")
                    if layer == 1:
                        s_cols = wpool.tile([P, CH], f32, tag="scols")
                    onehots = []
                    for c in range(CH):
                        oh = ohpool.tile([P, NPB], bf, tag="oh")
                        nc.vector.tensor_scalar(
                            out=oh[:], in0=iota_t[:, 0:NPB],
                            scalar1=st_slot[:, c:c + 1],
                            scalar2=None, op0=OP.is_equal)
                        onehots.append(oh)
                        ohT_ps = pstr.tile([NPB, P], bf, tag="tr")
                        nc.tensor.transpose(out=ohT_ps[:], in_=oh[:], identity=ident[:])
                        ohT = ohpool.tile([NPB, P], bf, tag="ohT")
                        nc.vector.tensor_copy(out=ohT[:], in_=ohT_ps[:])
                        if layer == 1:
                            xd_ps = pstr.tile([P, 128], f32, tag="tr")
                            nc.tensor.matmul(out=xd_ps[:], lhsT=ohT[:],
                                             rhs=dslots[:, 0:128],
                                             start=True, stop=True)
                            prod = wpool.tile([P, 128], f32, tag="prod")
                            nc.vector.tensor_tensor(out=prod[:], in0=g[:, c, 0:128],
                                                    in1=xd_ps[:], op=OP.mult)
                            nc.vector.tensor_reduce(out=s_cols[:, c:c + 1],
                                                    in_=prod[:], op=OP.add, axis=AX.X)
                        nc.tensor.matmul(out=adst_ps[:, c, :], lhsT=ohT[:],
                                         rhs=dslots[:, ad_off:ad_off + 8],
                                         start=True, stop=True)
                    if layer == 1:
                        p4b = wpool.tile([P, CH, 4], bf, tag="p4b")
                        nc.vector.tensor_copy(out=p4b[:, :, 0], in_=s_cols[:])
                        s2 = wpool.tile([P, CH], f32, tag="s2")
                        nc.vector.tensor_tensor(out=s2[:], in0=s_cols[:],
                                                in1=s_cols[:], op=OP.mult)
                        nc.vector.tensor_copy(out=p4b[:, :, 1], in_=s2[:])
                        s34 = wpool.tile([P, CH], f32, tag="s34")
                        nc.vector.tensor_tensor(out=s34[:], in0=s2[:], in1=s_cols[:],
                                                op=OP.mult)
                        nc.vector.tensor_copy(out=p4b[:, :, 2], in_=s34[:])
                        nc.vector.tensor_tensor(out=s34[:], in0=s2[:], in1=s2[:],
                                                op=OP.mult)
                        nc.vector.tensor_copy(out=p4b[:, :, 3], in_=s34[:])
                        ae_t = wpool.tile([P, CH, 16], bf, tag="aerow")
                        for c in range(CH):
                            p4T_ps = pstr.tile([4, P], bf, tag="tr")
                            nc.tensor.transpose(out=p4T_ps[:], in_=p4b[:, c, :],
                                                identity=ident[:])
                            p4T = spool.tile([4, P], bf, tag="p4T")
                            nc.vector.tensor_copy(out=p4T[:], in_=p4T_ps[:])
                            hid_ps = pstr.tile([16, P], f32, tag="tr")
                            nc.tensor.matmul(out=hid_ps[:], lhsT=w1e_t[:], rhs=p4T[:],
                                             start=True, stop=True)
                            hidr = spool.tile([16, P], bf, tag="hidr")
                            nc.vector.tensor_scalar(out=hidr[:], in0=hid_ps[:],
                                                    scalar1=eeb1_t[:], scalar2=0.0,
                                                    op0=OP.add, op1=OP.max)
                            aeT_ps = pstr.tile([16, P], f32, tag="tr")
                            nc.tensor.matmul(out=aeT_ps[:], lhsT=mb_t[:], rhs=hidr[:],
                                             start=True, stop=True)
                            aeT_b = spool.tile([16, P], bf, tag="aeTb")
                            nc.vector.tensor_copy(out=aeT_b[:], in_=aeT_ps[:])
                            ae_ps = pstr.tile([P, 16], bf, tag="tr")
                            nc.tensor.transpose(out=ae_ps[:], in_=aeT_b[:, 0:P],
                                                identity=ident[0:16, 0:16])
                            nc.vector.tensor_copy(out=ae_t[:, c, :], in_=ae_ps[:])
                        nc.sync.dma_start(
                            out=ae_dram[bass.ds(i, 1), :, :].rearrange(
                                "o p k -> (o p) k"),
                            in_=ae_t[:].rearrange("p c k -> p (c k)"))
                        ae_l = ae_t
                    else:
                        ae_l = wpool.tile([P, CH, 16], bf, tag="ael2")
                        nc.sync.dma_start(
                            out=ae_l[:].rearrange("p c k -> p (c k)"),
                            in_=ae_dram[bass.ds(i, 1), :, :].rearrange(
                                "o p k -> (o p) k"))
                    off = 0 if layer == 1 else 8
                    z = wpool.tile([P, CH, 8], f32, tag=f"z{layer}")
                    nc.vector.tensor_tensor(out=z[:], in0=g[:, :, as_off:as_off + 8],
                                            in1=adst_ps[:], op=OP.add)
                    nc.vector.tensor_tensor(out=z[:], in0=z[:],
                                            in1=ae_l[:, :, off:off + 8], op=OP.add)
                    if cfg["has_dd"]:
                        nc.vector.tensor_tensor(
                            out=z[:], in0=z[:],
                            in1=ddc_t[:, None, off:off + 8].to_broadcast([P, CH, 8]),
                            op=OP.add)
                    zz = wpool.tile([P, CH, 8], f32, tag=f"zz{layer}")
                    nc.vector.tensor_scalar(out=zz[:], in0=z[:], scalar1=NEG_SLOPE,
                                            scalar2=None, op0=OP.mult)
                    nc.vector.tensor_tensor(out=z[:], in0=z[:], in1=zz[:],
                                            op=OP.max)
                    mez = wpool.tile([P, CH, 136], bf, tag=f"mez{layer}")
                    nc.scalar.activation(out=mez[:, :, 128:136], in_=z[:],
                                         func=AF.Exp)
                    nc.vector.tensor_tensor(
                        out=mez[:, :, 0:128].rearrange("p c (h q) -> p c h q", h=8),
                        in0=g[:, :, xs_off:xs_off + 128].rearrange(
                            "p c (h q) -> p c h q", h=8),
                        in1=mez[:, :, 128:136, None].to_broadcast([P, CH, 8, 16]),
                        op=OP.mult)
                    for c in range(CH):
                        nc.tensor.matmul(out=aggp[:], lhsT=onehots[c][:],
                                         rhs=mez[:, c, :],
                                         start=(c == 0), stop=(c == CH - 1))
                    aggs = wpool.tile([NPB, 136], f32, tag=f"aggs{layer}")
                    nc.vector.tensor_copy(out=aggs[:], in_=aggp[:])
                    nc.sync.dma_start(out=agg_dst[bass.ds(i * NPB, NPB), :],
                                      in_=aggs[:])

                with nc.named_scope(scope):
                    tc.For_i_unrolled(0, T, 1, body, max_unroll=1)

            layer_pass(t1_full, t1_loc, 272, agg1, 1, "passA")

            # ================= node epilogue 1 (+T2 build) =================
            def epilogue(agg_src, hprev_loc, hres_loc, gln, bln, cb, scope, build_t2):
                def body(i):
                    ag = npool.tile([P, 136], f32, tag="ag")
                    nc.sync.dma_start(out=ag[:], in_=agg_src[bass.ds(i * P, P), :])
                    hp = npool.tile([P, 128], f32, tag="hp")
                    nc.sync.dma_start(out=hp[:], in_=hprev_loc[bass.ds(i * P, P), :])
                    rden = npool.tile([P, 8], f32, tag="rden")
                    nc.vector.reciprocal(out=rden[:], in_=ag[:, 128:136])
                    o1 = npool.tile([P, 128], f32, tag="o1")
                    nc.vector.tensor_tensor(
                        out=o1[:].rearrange("p (h q) -> p h q", h=8),
                        in0=ag[:, 0:128].rearrange("p (h q) -> p h q", h=8),
                        in1=rden[:, :, None].to_broadcast([P, 8, 16]), op=OP.mult)
                    if cb is not None:
                        nc.vector.tensor_tensor(out=o1[:], in0=o1[:], in1=cb,
                                                op=OP.add)
                    # elu(x) = max(x,0) + exp(min(x,0)) - 1, then + hprev
                    mn = npool.tile([P, 128], f32, tag="mn")
                    nc.vector.tensor_scalar(out=mn[:], in0=o1[:], scalar1=0.0,
                                            scalar2=None, op0=OP.min)
                    ex = npool.tile([P, 128], f32, tag="ex")
                    nc.scalar.activation(out=ex[:], in_=mn[:], func=AF.Exp)
                    mx = npool.tile([P, 128], f32, tag="mx")
                    nc.vector.tensor_scalar(out=mx[:], in0=o1[:], scalar1=0.0,
                                            scalar2=None, op0=OP.max)
                    h = npool.tile([P, 128], f32, tag="h")
                    nc.vector.tensor_tensor(out=h[:], in0=ex[:], in1=mx[:], op=OP.add)
                    nc.vector.tensor_scalar(out=h[:], in0=h[:], scalar1=-1.0,
                                            scalar2=None, op0=OP.add)
                    nc.vector.tensor_tensor(out=h[:], in0=h[:], in1=hp[:], op=OP.add)
                    # layernorm
                    msum = npool.tile([P, 1], f32, tag="msum")
                    nc.vector.tensor_reduce(out=msum[:], in_=h[:], op=OP.add, axis=AX.X)
                    mu = npool.tile([P, 1], f32, tag="mu")
                    nc.vector.tensor_scalar(out=mu[:], in0=msum[:],
                                            scalar1=1.0 / 128, scalar2=None,
                                            op0=OP.mult)
                    xc = npool.tile([P, 128], f32, tag="xc")
                    nc.vector.tensor_scalar(out=xc[:], in0=h[:], scalar1=mu[:],
                                            scalar2=None, op0=OP.subtract)
                    sq = npool.tile([P, 128], f32, tag="sq")
                    vsum = npool.tile([P, 1], f32, tag="vsum")
                    nc.scalar.activation(out=sq[:], in_=xc[:], func=AF.Square,
                                         accum_out=vsum[:])
                    sd = npool.tile([P, 1], f32, tag="sd")
                    nc.scalar.activation(out=sd[:], in_=vsum[:], func=AF.Sqrt,
                                         scale=sc128_t[:], bias=eps_t[:])
                    rsd = npool.tile([P, 1], f32, tag="rsd")
                    nc.vector.reciprocal(out=rsd[:], in_=sd[:])
                    hln = npool.tile([P, 128], f32, tag="hln")
                    nc.vector.tensor_scalar(out=hln[:], in0=xc[:], scalar1=rsd[:],
                                            scalar2=None, op0=OP.mult)
                    if gln is not None:
                        nc.vector.tensor_tensor(out=hln[:], in0=hln[:], in1=gln,
                                                op=OP.mult)
                    if bln is not None:
                        nc.vector.tensor_tensor(out=hln[:], in0=hln[:], in1=bln,
                                                op=OP.add)
                    if hres_loc is not None:
                        nc.sync.dma_start(out=hres_loc[bass.ds(i * P, P), :],
                                          in_=hln[:])
                    if build_t2:
                        hb = npool.tile([P, 128], bf, tag="hb")
                        nc.vector.tensor_copy(out=hb[:], in_=hln[:])
                        hT_ps = pstr.tile([P, P], bf, tag="tr")
                        nc.tensor.transpose(out=hT_ps[:], in_=hb[:], identity=ident[:])
                        hT = npool.tile([P, P], bf, tag="hT")
                        nc.vector.tensor_copy(out=hT[:], in_=hT_ps[:])
                        t2ps = pstr.tile([P, 144], f32, tag="tr")
                        nc.tensor.matmul(out=t2ps[:], lhsT=hT[:], rhs=wn2_t[:],
                                         start=True, stop=True)
                        t2row = npool.tile([P, 144], bf, tag="t2row")
                        nc.vector.tensor_copy(out=t2row[:], in_=t2ps[:])
                        nc.sync.dma_start(out=t2_loc[bass.ds(i * P, P), :],
                                          in_=t2row[:])
                    else:
                        # final head: JK + classifier + log_softmax
                        hh_ps = pshh.tile([P, 128], f32, tag="hh")
                        for l, hsrc in enumerate(("h0", "h1", "h2")):
                            if hsrc == "h2":
                                hsb = npool.tile([P, 128], bf, tag="hsb2")
                                nc.vector.tensor_copy(out=hsb[:], in_=hln[:])
                            else:
                                hsrc_loc = h0_loc if hsrc == "h0" else h1_loc
                                hsf = npool.tile([P, 128], f32, tag="hsf")
                                nc.sync.dma_start(
                                    out=hsf[:], in_=hsrc_loc[bass.ds(i * P, P), :])
                                hsb = npool.tile([P, 128], bf, tag="hsb")
                                nc.vector.tensor_copy(out=hsb[:], in_=hsf[:])
                            hsT_ps = pstr.tile([P, P], bf, tag="tr")
                            nc.tensor.transpose(out=hsT_ps[:], in_=hsb[:],
                                                identity=ident[:])
                            hsT = npool.tile([P, P], bf, tag="hsT")
                            nc.vector.tensor_copy(out=hsT[:], in_=hsT_ps[:])
                            nc.tensor.matmul(out=hh_ps[:], lhsT=hsT[:],
                                             rhs=jkw_t[:, l, :],
                                             start=(l == 0), stop=(l == 2))
                        hhb = npool.tile([P, 128], bf, tag="hhb")
                        nc.vector.tensor_copy(out=hhb[:], in_=hh_ps[:])
                        hhT_ps = pstr.tile([P, P], bf, tag="tr")
                        nc.tensor.transpose(out=hhT_ps[:], in_=hhb[:], identity=ident[:])
                        hhT = npool.tile([P, P], bf, tag="hhT")
                        nc.vector.tensor_copy(out=hhT[:], in_=hhT_ps[:])
                        lg_ps = pstr.tile([P, 40], f32, tag="tr")
                        nc.tensor.matmul(out=lg_ps[:], lhsT=hhT[:], rhs=clsw_t[:],
                                         start=True, stop=True)
                        lg = npool.tile([P, 40], f32, tag="lg")
                        if cfg["has_clsb"]:
                            nc.vector.tensor_tensor(out=lg[:], in0=lg_ps[:],
                                                    in1=clsb_t[:], op=OP.add)
                        else:
                            nc.vector.tensor_copy(out=lg[:], in_=lg_ps[:])
                        rmax = npool.tile([P, 1], f32, tag="rmax")
                        nc.vector.tensor_reduce(out=rmax[:], in_=lg[:], op=OP.max,
                                                axis=AX.X)
                        xm = npool.tile([P, 40], f32, tag="xm")
                        nc.vector.tensor_scalar(out=xm[:], in0=lg[:], scalar1=rmax[:],
                                                scalar2=None, op0=OP.subtract)
                        ee = npool.tile([P, 40], f32, tag="ee")
                        esum = npool.tile([P, 1], f32, tag="esum")
                        nc.scalar.activation(out=ee[:], in_=xm[:], func=AF.Exp,
                                             accum_out=esum[:])
                        lse = npool.tile([P, 1], f32, tag="lse")
                        nc.scalar.activation(out=lse[:], in_=esum[:], func=AF.Ln)
                        res = npool.tile([P, 40], f32, tag="res")
                        nc.vector.tensor_scalar(out=res[:], in0=xm[:], scalar1=lse[:],
                                                scalar2=None, op0=OP.subtract)
                        nc.sync.dma_start(out=out_t[bass.ds(i * P, P), :], in_=res[:])

                with nc.named_scope(scope):
                    tc.For_i_unrolled(0, NCH, 1, body, max_unroll=2)

            gln1 = bias_t[:, 3, :] if cfg["has_ln1g"] else None
            bln1 = bias_t[:, 4, :] if cfg["has_ln1b"] else None
            cb1 = bias_t[:, 1, :] if cfg["has_c1b"] else None
            epilogue(agg1, h0_loc, h1_loc, gln1, bln1, cb1, "ep1", True)

            with nc.named_scope("ag2"):
                if ncores > 1:
                    nc.gpsimd.collective_compute(
                        "AllGather", mybir.AluOpType.bypass,
                        replica_groups=[list(range(ncores))],
                        ins=[t2_loc[:]], outs=[t2_full[:]],
                    )
                else:
                    nc.sync.dma_start(out=t2_full[:], in_=t2_loc[:])

            layer_pass(t2_full, t2_loc, 144, agg2, 2, "passB")

            gln2 = bias_t[:, 5, :] if cfg["has_ln2g"] else None
            bln2 = bias_t[:, 6, :] if cfg["has_ln2b"] else None
            cb2 = bias_t[:, 2, :] if cfg["has_c2b"] else None
            epilogue(agg2, h1_loc, None, gln2, bln2, cb2, "ep2", False)

    nc.finalize()
    return nc


def _prepare(inputs):
    I = {k: np.asarray(v) for k, v in inputs.items()}
    x = I["x"].astype(np.float32)
    N = x.shape[0]
    NSH = N // NCORES
    T = (NSH + NPB - 1) // NPB
    NSHP = ((T * NPB + P - 1) // P) * P
    CAP, idxs = _preprocess(I["edge_index"], N, NSH, NSHP)
    W = _fold_weights(I)

    cfg = dict(
        N=N, NSH=NSH, NSHP=NSHP, CAP=CAP, ncores=NCORES,
        has_dd=bool(np.any(W["ddc"])),
        has_clsb=bool(np.any(W["clsb"])),
        has_ln1g=not np.allclose(I["n1_g"], 1.0),
        has_ln1b=bool(np.any(I["n1_b"])),
        has_ln2g=not np.allclose(I["n2_g"], 1.0),
        has_ln2b=bool(np.any(I["n2_b"])),
        has_c1b=bool(np.any(I["c1_bias"])),
        has_c2b=bool(np.any(I["c2_bias"])),
        has_inb=bool(np.any(I["in_b"])),
    )
    assert not cfg["has_inb"], "nonzero in_b not wired"
    iota = np.tile(np.arange(P, dtype=np.float32)[None, :], (P, 1))

    biases = np.zeros((P, 8, 128), np.float32)
    biases[:, 0] = W["inb"]
    biases[:, 1] = W["c1b"]
    biases[:, 2] = W["c2b"]
    biases[:, 3] = W["ln1g"]
    biases[:, 4] = W["ln1b"]
    biases[:, 5] = W["ln2g"]
    biases[:, 6] = W["ln2b"]

    common = {
        "iota_c": iota,
        "wnode1": _bf16_np(W["wnode1"]),
        "wnode2": _bf16_np(W["wnode2"]),
        "w1e": _bf16_np(W["w1e"]),
        "mboth": _bf16_np(W["mboth"]),
        "eeb1": W["eeb1"],
        "ddc": W["ddc"],
        "jkw": _bf16_np(W["jkw"]),
        "clsw": _bf16_np(W["clsw"]),
        "clsb": W["clsb"],
        "biases": biases,
    }
    in_maps = []
    for k in range(NCORES):
        n0 = k * cfg["NSH"]
        xsh = np.zeros((P, cfg["NSHP"]), np.float32)
        xsh[:, :cfg["NSH"]] = x[n0:n0 + cfg["NSH"]].T
        m = dict(common)
        m["xt"] = _bf16_np(xsh)
        m["src_idx"] = idxs[k][0]
        m["slot_f"] = idxs[k][1]
        in_maps.append(m)
    return cfg, in_maps


def _ensure_ntff_hook():
    """Register antenv.axon_hooks NTFF profile hook (dev-only, trace path).

    The agent image's antenv stub lacks axon_hooks, so boot degraded
    silently; bass_utils hard-imports it when trace=True under axon.
    Replicates trn_boot._ntff_profile_via_ctypes against the axon .so.
    """
    import contextlib
    import ctypes
    import sys
    import types

    try:
        from antenv.axon_hooks import get_axon_ntff_profile_hook  # noqa: F401
        return
    except ImportError:
        pass

    so_path = "/opt/axon/libaxon_pjrt.so"
    lib = ctypes.CDLL(so_path)
    if not hasattr(lib, "axon_start_nrt_profile"):
        return
    lib.axon_start_nrt_profile.argtypes = [
        ctypes.POINTER(ctypes.c_int64), ctypes.c_size_t]
    lib.axon_start_nrt_profile.restype = ctypes.c_int64
    lib.axon_stop_nrt_profile.argtypes = [ctypes.c_char_p]
    lib.axon_stop_nrt_profile.restype = ctypes.c_int64

    @contextlib.contextmanager
    def _hook(output_dir, device_ids):
        import jax
        jax.devices()
        if device_ids:
            ids = (ctypes.c_int64 * len(device_ids))(*device_ids)
            rc = lib.axon_start_nrt_profile(ids, len(device_ids))
        else:
            rc = lib.axon_start_nrt_profile(None, 0)
        if rc != 0:
            raise RuntimeError(f"axon_start_nrt_profile rc={rc}")
        try:
            yield
        finally:
            n = lib.axon_stop_nrt_profile(str(output_dir).encode())
            if n < 0:
                raise RuntimeError(f"axon_stop_nrt_profile rc={n}")
            print(f"profile: {n} file(s) written to {output_dir}")

    holder = {"h": _hook}
    mod = types.ModuleType("antenv.axon_hooks")
    mod.set_axon_ntff_profile_hook = lambda h: holder.__setitem__("h", h)
    mod.get_axon_ntff_profile_hook = lambda: holder.get("h")
    import antenv
    antenv.axon_hooks = mod
    sys.modules["antenv.axon_hooks"] = mod


def kernel(**inputs):
    global LAST_EXEC_NS, LAST_SCOPES
    from concourse.bass_utils import run_bass_kernel_spmd

    cfg, in_maps = _prepare(inputs)
    key = tuple(sorted(cfg.items()))
    if key not in _CACHE:
        _CACHE[key] = _build(cfg)
    nc = _CACHE[key]
    trace = bool(os.environ.get("KERNEL_TRACE"))
    kw = {}
    if trace:
        import tempfile
        try:
            _ensure_ntff_hook()
        except Exception:
            pass
        kw = dict(trace=True, tmpdir=tempfile.mkdtemp(prefix="ktrace_"))
    res = run_bass_kernel_spmd(nc, in_maps, core_ids=list(range(NCORES)), **kw)
    LAST_EXEC_NS = res.exec_time_ns
    LAST_SCOPES = res.per_core_scope_times
    NSH = cfg["NSH"]
    out = np.concatenate([res.results[k]["out"][:NSH] for k in range(NCORES)], axis=0)
    return out.astype(np.float32)



# revision 4
# speedup vs baseline: 1.0025x; 1.0025x over previous
"""Trainium2 Bass kernel v2 for nn_DepthAwareGATv2 (slot-major ELL, 8 cores).

Design vs baseline:
- Nodes per core sorted by in-degree (desc); tiles of 128 dst nodes with
  per-tile edge capacity K_t (global max over cores) -> ELL layout
  [128 nodes, K_t slots]. ~5% slot inflation instead of one-hot machinery.
- Gathered table row = [xs(128, qh-permuted) | asrc(8)] (272B). x is never
  gathered: dot products s = x_u.x_v/128 are computed as xs1_u . x~_v with
  x~ = x @ W1^{-T}/128 (local, built on device in P1).
- dst-side values (adst, x~) are per-partition -> broadcast via stride-0
  APs; no one-hots, no per-chunk transposes.
- Aggregation = K identity-lhsT matmuls accumulating in PSUM (f32).
- Edge MLP batched per 8-k groups via block-diagonal matmuls.
- Epilogue (softmax-div, elu, residual, LN) fused into the tile loop;
  rsqrt via exp(-0.5*ln(v)) so only one ACT table set (natural_log_exp).
- Padding edge slots point at the owner's pad table row whose asrc is set
  to -200 => exp(z)~0, no masks.
"""

import os

import numpy as np

NCORES = 8
P = 128
LAST_EXEC_NS = None
LAST_SCOPES = None

_CACHE = {}


def _bf16_np(a):
    import ml_dtypes
    return np.ascontiguousarray(
        np.asarray(a, dtype=np.float32).astype(ml_dtypes.bfloat16))


def _qh_perm(H, HID):
    # new col (q*H + h) <- old col (h*HID + q)
    return np.array([h * HID + q for q in range(HID) for h in range(H)],
                    dtype=np.int64)


def _preprocess(edge_index, N, NSH, NSHP):
    """ELL layout. Returns (Ks, st_idx per core, perms per core)."""
    src_all = edge_index[0].astype(np.int64)
    dst_all = edge_index[1].astype(np.int64)
    T = NSHP // P
    perms, invs, percore = [], [], []
    tile_max = np.zeros((NCORES, T), np.int64)
    for k in range(NCORES):
        m = (dst_all // NSH) == k
        src, dstl = src_all[m], dst_all[m] - k * NSH
        deg = np.bincount(dstl, minlength=NSH)
        perm = np.argsort(-deg, kind="stable")
        inv = np.empty(NSH, np.int64)
        inv[perm] = np.arange(NSH)
        perms.append(perm)
        invs.append(inv)
        degp = np.concatenate([deg[perm], np.zeros(NSHP - NSH, np.int64)])
        tile_max[k] = np.maximum(degp.reshape(T, P).max(1), 1)
        percore.append((src, dstl, deg))
    Ks = tile_max.max(0)                     # global per-tile capacity
    koff = np.zeros(T + 1, np.int64)
    np.cumsum(Ks, out=koff[1:])
    SK = int(koff[-1])
    idxs = []
    for k in range(NCORES):
        src, dstl, deg = percore[k]
        inv = invs[k]
        pos = inv[dstl]
        order = np.argsort(pos, kind="stable")
        pos_s = pos[order]
        src_s = src[order]
        degp = deg[perms[k]]
        starts = np.zeros(NSH + 1, np.int64)
        np.cumsum(degp, out=starts[1:])
        kidx = np.arange(len(pos_s)) - starts[pos_s]
        owner = src_s // NSH
        rows = owner * NSHP + np.concatenate(
            [invs[j] for j in range(NCORES)])[src_s % NSH + owner * NSH]
        pad_row = k * NSHP + (NSHP - 1)
        st = np.full((P, SK), pad_row, np.int32)
        t_arr = pos_s // P
        p_arr = pos_s % P
        st[p_arr, koff[t_arr] + kidx] = rows.astype(np.int32)
        idxs.append(st)
    return Ks, koff, idxs, perms


def _fold_weights(I):
    H, HID = I["c1_att_src"].shape           # 8, 16
    F = I["x"].shape[1]
    D = H * HID
    sig = _qh_perm(H, HID)

    def v(lin_w, att):
        return (lin_w.reshape(lin_w.shape[0], H, HID) * att[None]).sum(-1)

    W = {}
    W1 = I["c1_lin_w"].astype(np.float64)
    Wtil = np.linalg.inv(W1).T / 128.0       # x~ = x @ Wtil
    V1s, V1d = v(I["c1_lin_w"], I["c1_att_src"]), v(I["c1_lin_w"], I["c1_att_dst"])
    V2s, V2d = v(I["c2_lin_w"], I["c2_att_src"]), v(I["c2_lin_w"], I["c2_att_dst"])
    G1 = np.einsum("jhc,hc->jh", I["c1_edge_w"].reshape(H, H, HID), I["c1_att_edge"])
    G2 = np.einsum("jhc,hc->jh", I["c2_edge_w"].reshape(H, H, HID), I["c2_att_edge"])
    mb = np.concatenate([I["ee_w2"] @ G1, I["ee_w2"] @ G2], axis=1)   # [16,16]
    dd = np.concatenate([I["ee_b2"] @ G1, I["ee_b2"] @ G2])           # [16]

    # wn1 cols: xs1p(128) asrc1(8) x~p(128) adst1(8) h0p(128) = 400
    W["wn1"] = np.concatenate(
        [I["c1_lin_w"][:, sig], V1s, Wtil[:, sig].astype(np.float32), V1d,
         I["in_w"][:, sig]], axis=1)
    b1 = np.zeros((1, 400), np.float32)
    b1[0, 264:272] = dd[0:8]
    b1[0, 272:400] = I["in_b"][sig]
    W["b1row"] = b1
    # wn2 cols: xs2p(128) asrc2(8) adst2(8) = 144 ; rows are h1p dims
    W["wn2"] = np.concatenate(
        [I["c2_lin_w"][sig][:, sig], V2s[sig], V2d[sig]], axis=1)
    b2 = np.zeros((1, 144), np.float32)
    b2[0, 136:144] = dd[8:16]
    W["b2row"] = b2
    # edge MLP blockdiags: p4T rows (k*4+pow), hid cols (k*16+j)
    w1e = I["ee_w1"]                                   # [4,16]
    bdw1 = np.zeros((32, 128), np.float32)
    bdmb = np.zeros((128, 128), np.float32)
    for kk in range(8):
        bdw1[kk * 4:kk * 4 + 4, kk * 16:kk * 16 + 16] = w1e
        bdmb[kk * 16:kk * 16 + 16, kk * 16:kk * 16 + 16] = mb
    W["bdw1"], W["bdmb"] = bdw1, bdmb
    W["eeb1rep"] = np.tile(I["ee_b1"], 8)[:, None].astype(np.float32)  # [128,1]
    jkw = np.stack([I["jk_w"][0:D][sig], I["jk_w"][D:2 * D][sig],
                    I["jk_w"][2 * D:3 * D][sig]], axis=1)              # [128,3,128]
    W["jkw"] = jkw
    W["clsw"] = I["cls_w"]
    clsb = I["cls_b"] + I["jk_b"] @ I["cls_w"]
    W["clsbrow"] = clsb[None, :].astype(np.float32)                    # [1,40]
    W["c1brep"] = np.tile(I["c1_bias"][sig][None], (P, 1)).astype(np.float32)
    W["c2brep"] = np.tile(I["c2_bias"][sig][None], (P, 1)).astype(np.float32)
    W["ln1g"] = np.tile(I["n1_g"][sig][None], (P, 1)).astype(np.float32)
    W["ln1b"] = np.tile(I["n1_b"][sig][None], (P, 1)).astype(np.float32)
    W["ln2g"] = np.tile(I["n2_g"][sig][None], (P, 1)).astype(np.float32)
    W["ln2b"] = np.tile(I["n2_b"][sig][None], (P, 1)).astype(np.float32)
    return W


def _build(cfg):
    import concourse.bass as bass
    import concourse.mybir as mybir
    from concourse.bacc import Bacc
    from concourse.tile import TileContext
    from concourse.masks import make_identity

    f32 = mybir.dt.float32
    bf = mybir.dt.bfloat16
    i32 = mybir.dt.int32
    AF = mybir.ActivationFunctionType
    OP = mybir.AluOpType
    AX = mybir.AxisListType

    NSHP, ncores = cfg["NSHP"], cfg["ncores"]
    Ks = cfg["Ks"]
    T = len(Ks)
    KM = max(Ks)
    SK = sum(Ks)
    koff = [0]
    for kk in Ks:
        koff.append(koff[-1] + kk)
    NCH = NSHP // P
    NG = NSHP * ncores
    NPAD0 = cfg["NSH"]                      # first pad row

    nc = Bacc(num_devices=ncores)

    xt = nc.dram_tensor("xt", [P, NSHP], bf, kind="ExternalInput")
    st_idx = nc.dram_tensor("st_idx", [P, SK], i32, kind="ExternalInput")
    wn1 = nc.dram_tensor("wn1", [P, 400], bf, kind="ExternalInput")
    b1row = nc.dram_tensor("b1row", [1, 400], bf, kind="ExternalInput")
    wn2 = nc.dram_tensor("wn2", [P, 144], bf, kind="ExternalInput")
    b2row = nc.dram_tensor("b2row", [1, 144], bf, kind="ExternalInput")
    bdw1 = nc.dram_tensor("bdw1", [32, P], bf, kind="ExternalInput")
    bdmb = nc.dram_tensor("bdmb", [P, P], bf, kind="ExternalInput")
    jkw = nc.dram_tensor("jkw", [P, 3, P], bf, kind="ExternalInput")
    clsw = nc.dram_tensor("clsw", [P, 40], bf, kind="ExternalInput")
    clsbrow = nc.dram_tensor("clsbrow", [1, 40], bf, kind="ExternalInput")
    extras = nc.dram_tensor("extras", [P, 7, P], f32, kind="ExternalInput")
    # extras idx: 0 c1brep 1 c2brep 2 ln1g 3 ln1b 4 ln2g 5 ln2b 6 eeb1rep(col0)

    out_t = nc.dram_tensor("out", [NSHP, 40], f32, kind="ExternalOutput")

    with TileContext(nc) as tc:
        with (
            tc.tile_pool(name="dram", bufs=1, space="DRAM") as dpool,
            tc.tile_pool(name="consts", bufs=1) as cpool,
            tc.tile_pool(name="gather", bufs=2) as gpool,
            tc.tile_pool(name="mezp", bufs=2) as mpool,
            tc.tile_pool(name="work", bufs=2) as wpool,
            tc.tile_pool(name="small", bufs=3) as spool,
            tc.tile_pool(name="pstr", bufs=2, space="PSUM") as pstr,
            tc.tile_pool(name="psmm", bufs=2, space="PSUM") as psmm,
            tc.tile_pool(name="psacc", bufs=2, space="PSUM") as psacc,
        ):
            shared = {"addr_space": "Shared"} if ncores > 1 else {}
            t1_loc = dpool.tile([NSHP, 136], bf, name="t1_loc")
            t1_full = dpool.tile([NG, 136], bf, name="t1_full", **shared)
            t2_loc = dpool.tile([NSHP, 136], bf, name="t2_loc")
            t2_full = dpool.tile([NG, 136], bf, name="t2_full", **shared)
            loc1 = dpool.tile([NSHP, 136], bf, name="loc1")
            loc2 = dpool.tile([NSHP, 8], bf, name="loc2")
            h0_loc = dpool.tile([NSHP, P], f32, name="h0_loc")
            h1_loc = dpool.tile([NSHP, P], f32, name="h1_loc")
            ae_dram = dpool.tile([T, P, KM * 16], bf, name="ae_dram")

            def ld(pool, shape, dt_, src):
                t = pool.tile(shape, dt_, name=f"c_{src.name}")
                nc.sync.dma_start(out=t[:], in_=src[:])
                return t

            wn1_t = ld(cpool, [P, 400], bf, wn1)
            b1_t = ld(cpool, [1, 400], bf, b1row)
            wn2_t = ld(cpool, [P, 144], bf, wn2)
            b2_t = ld(cpool, [1, 144], bf, b2row)
            bdw1_t = ld(cpool, [32, P], bf, bdw1)
            bdmb_t = ld(cpool, [P, P], bf, bdmb)
            jkw_t = ld(cpool, [P, 3, P], bf, jkw)
            clsw_t = ld(cpool, [P, 40], bf, clsw)
            clsb_t = ld(cpool, [1, 40], bf, clsbrow)
            ex_t = ld(cpool, [P, 7, P], f32, extras) if cfg["any_extra"] else None
            ident = cpool.tile([P, P], bf, name="ident")
            make_identity(nc, ident[:])
            ones1 = cpool.tile([1, P], bf, name="ones1")
            nc.vector.memset(ones1[:], 1.0)
            eps_t = cpool.tile([P, 1], f32, name="eps_t")
            nc.vector.memset(eps_t[:], 1e-5)
            padfix = cpool.tile([NSHP - NPAD0, 8], bf, name="padfix")
            nc.vector.memset(padfix[:], -200.0)

            # ================= P1: node tables + h0 =================
            with nc.named_scope("p1"):
                for i in range(NCH):
                    xchunk = spool.tile([P, P], bf, tag="xchunk")
                    nc.sync.dma_start(out=xchunk[:], in_=xt[:, bass.ds(i * P, P)])
                    ps = psmm.tile([P, 400], f32, tag="mm")
                    nc.tensor.matmul(out=ps[:], lhsT=ones1[:], rhs=b1_t[:],
                                     start=True, stop=False)
                    nc.tensor.matmul(out=ps[:], lhsT=xchunk[:], rhs=wn1_t[:],
                                     start=False, stop=True)
                    t1row = spool.tile([P, 136], bf, tag="t1row")
                    nc.vector.tensor_copy(out=t1row[:], in_=ps[:, 0:136])
                    nc.sync.dma_start(out=t1_loc[bass.ds(i * P, P), :], in_=t1row[:])
                    l1row = spool.tile([P, 136], bf, tag="l1row")
                    nc.vector.tensor_copy(out=l1row[:], in_=ps[:, 136:272])
                    nc.sync.dma_start(out=loc1[bass.ds(i * P, P), :], in_=l1row[:])
                    h0row = spool.tile([P, P], f32, tag="h0row")
                    nc.scalar.copy(out=h0row[:], in_=ps[:, 272:400])
                    nc.sync.dma_start(out=h0_loc[bass.ds(i * P, P), :], in_=h0row[:])
                # pad rows: asrc := -200
                nc.sync.dma_start(
                    out=t1_loc[bass.ds(NPAD0, NSHP - NPAD0), 128:136],
                    in_=padfix[:])

            with nc.named_scope("ag1"):
                if ncores > 1:
                    nc.gpsimd.collective_compute(
                        "AllGather", mybir.AluOpType.bypass,
                        replica_groups=[list(range(ncores))],
                        ins=[t1_loc[:]], outs=[t1_full[:]],
                    )
                else:
                    nc.sync.dma_start(out=t1_full[:], in_=t1_loc[:])

            def layer_pass(layer, tbl, scope):
                aoff = 0 if layer == 1 else 8
                for t in range(T):
                    K = int(Ks[t])
                    G = (K + 7) // 8
                    r0 = t * P
                    with nc.named_scope(f"{scope}_g"):
                        idx_t = spool.tile([P, KM], i32, tag=f"idx{layer}")
                        nc.sync.dma_start(
                            out=idx_t[:, 0:K],
                            in_=st_idx[:, bass.ds(koff[t], K)])
                        g = gpool.tile([P, KM, 136], bf, tag=f"g{layer}")
                        if cfg["multi_offset"]:
                            nc.gpsimd.indirect_dma_start(
                                out=g[:, 0:K, :], out_offset=None, in_=tbl[:],
                                in_offset=bass.IndirectOffsetOnAxis(
                                    ap=idx_t[:, 0:K], axis=0))
                        else:
                            for k in range(K):
                                nc.gpsimd.indirect_dma_start(
                                    out=g[:, k, :], out_offset=None, in_=tbl[:],
                                    in_offset=bass.IndirectOffsetOnAxis(
                                        ap=idx_t[:, k:k + 1], axis=0))
                    if layer == 1:
                        lt = spool.tile([P, 136], bf, tag="l1t")
                        nc.sync.dma_start(out=lt[:], in_=loc1[bass.ds(r0, P), :])
                        hprev = spool.tile([P, P], f32, tag="h0t")
                        nc.sync.dma_start(out=hprev[:], in_=h0_loc[bass.ds(r0, P), :])
                    else:
                        lt = spool.tile([P, 8], bf, tag="l2t")
                        nc.sync.dma_start(out=lt[:], in_=loc2[bass.ds(r0, P), :])
                        hprev = spool.tile([P, P], f32, tag="h1t")
                        nc.sync.dma_start(out=hprev[:], in_=h1_loc[bass.ds(r0, P), :])
                        h0t = spool.tile([P, P], f32, tag="h0tB")
                        nc.sync.dma_start(out=h0t[:], in_=h0_loc[bass.ds(r0, P), :])

                    ae = wpool.tile([P, KM, 16], bf, tag="ae")
                    if layer == 1:
                        # --- dots ---
                        s_t = wpool.tile([P, KM], f32, tag="s_t")
                        scr = wpool.tile([P, P], bf, tag="scr")
                        scr2 = wpool.tile([P, P], bf, tag="scr2")
                        for k in range(K):
                            eng = (nc.gpsimd if (cfg["gpoff"] and k % 2)
                                   else nc.vector)
                            if cfg["use_ttr"]:
                                nc.vector.tensor_tensor_reduce(
                                    out=scr[:], in0=g[:, k, 0:128], in1=lt[:, 0:128],
                                    scale=1.0, scalar=0.0, op0=OP.mult, op1=OP.add,
                                    accum_out=s_t[:, k:k + 1])
                            else:
                                sc2 = (scr if eng is nc.vector else scr2)
                                eng.tensor_tensor(
                                    out=sc2[:], in0=g[:, k, 0:128], in1=lt[:, 0:128],
                                    op=OP.mult)
                                nc.vector.tensor_reduce(
                                    out=s_t[:, k:k + 1], in_=sc2[:], op=OP.add,
                                    axis=AX.X)
                        # --- poly powers [s, s2, s3, s4] ---
                        p4 = wpool.tile([P, KM, 4], bf, tag="p4")
                        nc.vector.tensor_copy(out=p4[:, 0:K, 0], in_=s_t[:, 0:K])
                        nc.vector.tensor_tensor(out=p4[:, 0:K, 1], in0=p4[:, 0:K, 0],
                                                in1=p4[:, 0:K, 0], op=OP.mult)
                        nc.vector.tensor_tensor(out=p4[:, 0:K, 2], in0=p4[:, 0:K, 1],
                                                in1=p4[:, 0:K, 0], op=OP.mult)
                        nc.vector.tensor_tensor(out=p4[:, 0:K, 3], in0=p4[:, 0:K, 1],
                                                in1=p4[:, 0:K, 1], op=OP.mult)
                        # --- edge MLP in 8-k blocks ---
                        for gi in range(G):
                            kk = min(8, K - gi * 8)
                            p4T_ps = pstr.tile([32, P], bf, tag="tr")
                            nc.tensor.transpose(
                                out=p4T_ps[0:4 * kk, :],
                                in_=p4[:, gi * 8:gi * 8 + kk, :].rearrange(
                                    "p k f -> p (k f)"),
                                identity=ident[:])
                            p4T = spool.tile([32, P], bf, tag="p4T")
                            nc.vector.tensor_copy(out=p4T[0:4 * kk, :],
                                                  in_=p4T_ps[0:4 * kk, :])
                            hid_ps = psmm.tile([P, P], f32, tag="mm")
                            nc.tensor.matmul(out=hid_ps[:], lhsT=bdw1_t[0:4 * kk, :],
                                             rhs=p4T[0:4 * kk, :],
                                             start=True, stop=True)
                            hidT = spool.tile([P, P], bf, tag="hidT")
                            if cfg["has_eeb1"]:
                                nc.scalar.activation(
                                    out=hidT[:], in_=hid_ps[:], func=AF.Relu,
                                    bias=ex_t[:, 6, 0:1])
                            else:
                                nc.scalar.activation(out=hidT[:], in_=hid_ps[:],
                                                     func=AF.Relu)
                            ae_ps = psmm.tile([P, P], f32, tag="mm")
                            nc.tensor.matmul(out=ae_ps[:], lhsT=bdmb_t[:],
                                             rhs=hidT[:], start=True, stop=True)
                            aeT = spool.tile([P, P], bf, tag="aeT")
                            nc.scalar.copy(out=aeT[:], in_=ae_ps[:])
                            aeb_ps = pstr.tile([P, P], bf, tag="tr")
                            nc.tensor.transpose(out=aeb_ps[:], in_=aeT[:],
                                                identity=ident[:])
                            nc.vector.tensor_copy(
                                out=ae[:, gi * 8:gi * 8 + kk, :].rearrange(
                                    "p k f -> p (k f)"),
                                in_=aeb_ps[:, 0:16 * kk])
                        nc.sync.dma_start(
                            out=ae_dram[bass.ds(t, 1), :, 0:K * 16].rearrange(
                                "o p c -> (o p) c"),
                            in_=ae[:, 0:K, :].rearrange("p k f -> p (k f)"))
                    else:
                        nc.sync.dma_start(
                            out=ae[:, 0:K, :].rearrange("p k f -> p (k f)"),
                            in_=ae_dram[bass.ds(t, 1), :, 0:K * 16].rearrange(
                                "o p c -> (o p) c"))

                    # --- z chain (bf16) ---
                    z1 = wpool.tile([P, KM, 8], bf, tag="z1")
                    nc.vector.tensor_tensor(out=z1[:, 0:K, :], in0=g[:, 0:K, 128:136],
                                            in1=ae[:, 0:K, aoff:aoff + 8], op=OP.add)
                    adst_ap = (lt[:, None, 128:136] if layer == 1
                               else lt[:, None, 0:8]).to_broadcast([P, K, 8])
                    nc.vector.tensor_tensor(out=z1[:, 0:K, :], in0=z1[:, 0:K, :],
                                            in1=adst_ap, op=OP.add)
                    zl = wpool.tile([P, KM, 8], bf, tag="zl")
                    if cfg["use_stt"]:
                        nc.vector.scalar_tensor_tensor(
                            out=zl[:, 0:K, :], in0=z1[:, 0:K, :], scalar=0.2,
                            in1=z1[:, 0:K, :], op0=OP.mult, op1=OP.max)
                    else:
                        zeng = nc.gpsimd if cfg["gpoff"] else nc.vector
                        zeng.tensor_scalar(out=zl[:, 0:K, :], in0=z1[:, 0:K, :],
                                           scalar1=0.2, scalar2=None,
                                           op0=OP.mult)
                        zeng.tensor_tensor(out=zl[:, 0:K, :], in0=zl[:, 0:K, :],
                                           in1=z1[:, 0:K, :], op=OP.max)
                    # --- w = exp(z); mez = xs * w (qh layout) ---
                    mezw = mpool.tile([P, KM, 136], bf, tag="mezw")
                    nc.scalar.activation(out=mezw[:, 0:K, 128:136], in_=zl[:, 0:K, :],
                                         func=AF.Exp)
                    nc.vector.tensor_tensor(
                        out=mezw[:, 0:K, 0:128].rearrange(
                            "p k (q h) -> p k q h", q=16),
                        in0=g[:, 0:K, 0:128].rearrange("p k (q h) -> p k q h", q=16),
                        in1=mezw[:, 0:K, None, 128:136].to_broadcast([P, K, 16, 8]),
                        op=OP.mult)
                    # --- aggregate over k on PE (f32 PSUM) ---
                    aggp = psacc.tile([P, 136], f32, tag="agg")
                    for k in range(K):
                        nc.tensor.matmul(out=aggp[:], lhsT=ident[:],
                                         rhs=mezw[:, k, :],
                                         start=(k == 0), stop=(k == K - 1))
                    # --- epilogue ---
                    rden = wpool.tile([P, 8], f32, tag="rden")
                    nc.vector.reciprocal(out=rden[:], in_=aggp[:, 128:136])
                    o1 = wpool.tile([P, P], f32, tag="o1")
                    nc.vector.tensor_tensor(
                        out=o1[:].rearrange("p (q h) -> p q h", q=16),
                        in0=aggp[:, 0:128].rearrange("p (q h) -> p q h", q=16),
                        in1=rden[:, None, :].to_broadcast([P, 16, 8]), op=OP.mult)
                    cb = cfg["has_c1b"] if layer == 1 else cfg["has_c2b"]
                    if cb:
                        nc.vector.tensor_tensor(
                            out=o1[:], in0=o1[:],
                            in1=ex_t[:, 0 if layer == 1 else 1, :], op=OP.add)
                    ex = wpool.tile([P, P], f32, tag="ex")
                    nc.scalar.activation(out=ex[:], in_=o1[:], func=AF.Exp)
                    tmp = wpool.tile([P, P], f32, tag="tmp")
                    h = wpool.tile([P, P], f32, tag="h")
                    if cfg["use_stt"]:
                        nc.vector.scalar_tensor_tensor(
                            out=tmp[:], in0=ex[:], scalar=1.0, in1=hprev[:],
                            op0=OP.min, op1=OP.add)
                        nc.vector.scalar_tensor_tensor(
                            out=h[:], in0=o1[:], scalar=0.0, in1=tmp[:],
                            op0=OP.max, op1=OP.add)
                    else:
                        eeng = nc.gpsimd if cfg["gpoff"] else nc.vector
                        eeng.tensor_scalar(out=tmp[:], in0=ex[:], scalar1=1.0,
                                           scalar2=None, op0=OP.min)
                        eeng.tensor_tensor(out=tmp[:], in0=tmp[:], in1=hprev[:],
                                           op=OP.add)
                        eeng.tensor_scalar(out=h[:], in0=o1[:], scalar1=0.0,
                                           scalar2=None, op0=OP.max)
                        eeng.tensor_tensor(out=h[:], in0=h[:], in1=tmp[:],
                                           op=OP.add)
                    # layernorm (no sqrt table: rstd = exp(-0.5 ln(v+eps)))
                    hsum = wpool.tile([P, 1], f32, tag="hsum")
                    nc.vector.tensor_reduce(out=hsum[:], in_=h[:], op=OP.add,
                                            axis=AX.X)
                    mu = wpool.tile([P, 1], f32, tag="mu")
                    nc.vector.tensor_scalar(out=mu[:], in0=hsum[:],
                                            scalar1=1.0 / 128, scalar2=None,
                                            op0=OP.mult)
                    xc = wpool.tile([P, P], f32, tag="xc")
                    nc.vector.tensor_scalar(out=xc[:], in0=h[:], scalar1=mu[:],
                                            scalar2=None, op0=OP.subtract)
                    sq = wpool.tile([P, P], bf, tag="sqscr")
                    vsum = wpool.tile([P, 1], f32, tag="vsum")
                    if cfg["use_ttr"]:
                        nc.vector.tensor_tensor_reduce(
                            out=sq[:], in0=xc[:], in1=xc[:], scale=1.0, scalar=0.0,
                            op0=OP.mult, op1=OP.add, accum_out=vsum[:])
                    else:
                        nc.vector.tensor_tensor(out=sq[:], in0=xc[:], in1=xc[:],
                                                op=OP.mult)
                        nc.vector.tensor_reduce(out=vsum[:], in_=sq[:], op=OP.add,
                                                axis=AX.X)
                    lnv = wpool.tile([P, 1], f32, tag="lnv")
                    nc.scalar.activation(out=lnv[:], in_=vsum[:], func=AF.Ln,
                                         scale=1.0 / 128, bias=eps_t[:])
                    rstd = wpool.tile([P, 1], f32, tag="rstd")
                    nc.scalar.activation(out=rstd[:], in_=lnv[:], func=AF.Exp,
                                         scale=-0.5)
                    hln = wpool.tile([P, P], f32, tag="hln")
                    nc.vector.tensor_scalar(out=hln[:], in0=xc[:], scalar1=rstd[:],
                                            scalar2=None, op0=OP.mult)
                    gl = cfg["has_ln1g"] if layer == 1 else cfg["has_ln2g"]
                    bl = cfg["has_ln1b"] if layer == 1 else cfg["has_ln2b"]
                    if gl:
                        nc.vector.tensor_tensor(
                            out=hln[:], in0=hln[:],
                            in1=ex_t[:, 2 if layer == 1 else 4, :], op=OP.mult)
                    if bl:
                        nc.vector.tensor_tensor(
                            out=hln[:], in0=hln[:],
                            in1=ex_t[:, 3 if layer == 1 else 5, :], op=OP.add)

                    if layer == 1:
                        nc.sync.dma_start(out=h1_loc[bass.ds(r0, P), :], in_=hln[:])
                        h1b = wpool.tile([P, P], bf, tag="h1b")
                        nc.vector.tensor_copy(out=h1b[:], in_=hln[:])
                        h1T_ps = pstr.tile([P, P], bf, tag="tr")
                        nc.tensor.transpose(out=h1T_ps[:], in_=h1b[:],
                                            identity=ident[:])
                        h1T = spool.tile([P, P], bf, tag="h1T")
                        nc.vector.tensor_copy(out=h1T[:], in_=h1T_ps[:])
                        t2ps = psmm.tile([P, 144], f32, tag="mm")
                        nc.tensor.matmul(out=t2ps[:], lhsT=ones1[:], rhs=b2_t[:],
                                         start=True, stop=False)
                        nc.tensor.matmul(out=t2ps[:], lhsT=h1T[:], rhs=wn2_t[:],
                                         start=False, stop=True)
                        t2row = spool.tile([P, 136], bf, tag="t2row")
                        nc.vector.tensor_copy(out=t2row[:], in_=t2ps[:, 0:136])
                        nc.sync.dma_start(out=t2_loc[bass.ds(r0, P), :], in_=t2row[:])
                        l2row = spool.tile([P, 8], bf, tag="l2row")
                        nc.vector.tensor_copy(out=l2row[:], in_=t2ps[:, 136:144])
                        nc.sync.dma_start(out=loc2[bass.ds(r0, P), :], in_=l2row[:])
                    else:
                        # final head: JK + classifier + log_softmax
                        hh_ps = psacc.tile([P, P], f32, tag="hh")
                        for li, hsrc in enumerate((h0t, hprev, hln)):
                            hsb = spool.tile([P, P], bf, tag="hsb")
                            nc.vector.tensor_copy(out=hsb[:], in_=hsrc[:])
                            hsT_ps = pstr.tile([P, P], bf, tag="tr")
                            nc.tensor.transpose(out=hsT_ps[:], in_=hsb[:],
                                                identity=ident[:])
                            hsT = spool.tile([P, P], bf, tag="hsT")
                            nc.vector.tensor_copy(out=hsT[:], in_=hsT_ps[:])
                            nc.tensor.matmul(out=hh_ps[:], lhsT=hsT[:],
                                             rhs=jkw_t[:, li, :],
                                             start=(li == 0), stop=(li == 2))
                        hhb = spool.tile([P, P], bf, tag="hhb")
                        nc.vector.tensor_copy(out=hhb[:], in_=hh_ps[:])
                        hhT_ps = pstr.tile([P, P], bf, tag="tr")
                        nc.tensor.transpose(out=hhT_ps[:], in_=hhb[:],
                                            identity=ident[:])
                        hhT = spool.tile([P, P], bf, tag="hhT")
                        nc.vector.tensor_copy(out=hhT[:], in_=hhT_ps[:])
                        lg_ps = psmm.tile([P, 40], f32, tag="mm")
                        nc.tensor.matmul(out=lg_ps[:], lhsT=ones1[:], rhs=clsb_t[:],
                                         start=True, stop=False)
                        nc.tensor.matmul(out=lg_ps[:], lhsT=hhT[:], rhs=clsw_t[:],
                                         start=False, stop=True)
                        rmax = wpool.tile([P, 1], f32, tag="rmax")
                        nc.vector.tensor_reduce(out=rmax[:], in_=lg_ps[:], op=OP.max,
                                                axis=AX.X)
                        xm = wpool.tile([P, 40], f32, tag="xm")
                        nc.vector.tensor_scalar(out=xm[:], in0=lg_ps[:],
                                                scalar1=rmax[:], scalar2=None,
                                                op0=OP.subtract)
                        ee = wpool.tile([P, 40], f32, tag="eeexp")
                        esum = wpool.tile([P, 1], f32, tag="esum")
                        nc.scalar.activation(out=ee[:], in_=xm[:], func=AF.Exp,
                                             accum_out=esum[:])
                        lse = wpool.tile([P, 1], f32, tag="lse")
                        nc.scalar.activation(out=lse[:], in_=esum[:], func=AF.Ln)
                        res = wpool.tile([P, 40], f32, tag="res")
                        nc.vector.tensor_scalar(out=res[:], in0=xm[:], scalar1=lse[:],
                                                scalar2=None, op0=OP.subtract)
                        nc.sync.dma_start(out=out_t[bass.ds(r0, P), :], in_=res[:])

            with nc.named_scope("passA"):
                layer_pass(1, t1_full, "pA")
                # pad rows of t2: asrc := -200 (xs2 already 0 since h1_pad = 0)
                nc.sync.dma_start(
                    out=t2_loc[bass.ds(NPAD0, NSHP - NPAD0), 128:136],
                    in_=padfix[:])

            with nc.named_scope("ag2"):
                if ncores > 1:
                    nc.gpsimd.collective_compute(
                        "AllGather", mybir.AluOpType.bypass,
                        replica_groups=[list(range(ncores))],
                        ins=[t2_loc[:]], outs=[t2_full[:]],
                    )
                else:
                    nc.sync.dma_start(out=t2_full[:], in_=t2_loc[:])

            with nc.named_scope("passB"):
                layer_pass(2, t2_full, "pB")

    nc.finalize()
    return nc


def _prepare(inputs):
    I = {k: np.asarray(v) for k, v in inputs.items()}
    x = I["x"].astype(np.float32)
    N = x.shape[0]
    NSH = N // NCORES
    NSHP = ((NSH + P - 1) // P) * P
    Ks, koff, idxs, perms = _preprocess(I["edge_index"], N, NSH, NSHP)
    W = _fold_weights(I)

    cfg = dict(
        N=N, NSH=NSH, NSHP=NSHP, ncores=NCORES, Ks=tuple(int(k) for k in Ks),
        multi_offset=bool(int(os.environ.get("K2_MULTI_OFFSET", "0"))),
        use_ttr=bool(int(os.environ.get("K2_TTR", "0"))),
        use_stt=bool(int(os.environ.get("K2_STT", "0"))),
        gpoff=bool(int(os.environ.get("K2_GPOFF", "0"))),
        has_eeb1=bool(np.any(I["ee_b1"])),
        has_c1b=bool(np.any(I["c1_bias"])),
        has_c2b=bool(np.any(I["c2_bias"])),
        has_ln1g=not np.allclose(I["n1_g"], 1.0),
        has_ln1b=bool(np.any(I["n1_b"])),
        has_ln2g=not np.allclose(I["n2_g"], 1.0),
        has_ln2b=bool(np.any(I["n2_b"])),
    )
    cfg["any_extra"] = (cfg["has_c1b"] or cfg["has_c2b"] or cfg["has_ln1g"]
                        or cfg["has_ln1b"] or cfg["has_ln2g"] or cfg["has_ln2b"]
                        or cfg["has_eeb1"])

    extras = np.zeros((P, 7, P), np.float32)
    extras[:, 0] = W["c1brep"]
    extras[:, 1] = W["c2brep"]
    extras[:, 2] = W["ln1g"]
    extras[:, 3] = W["ln1b"]
    extras[:, 4] = W["ln2g"]
    extras[:, 5] = W["ln2b"]
    extras[:, 6, 0:1] = W["eeb1rep"][:, 0:1] * 0 + W["eeb1rep"]

    common = {
        "wn1": _bf16_np(W["wn1"]), "b1row": _bf16_np(W["b1row"]),
        "wn2": _bf16_np(W["wn2"]), "b2row": _bf16_np(W["b2row"]),
        "bdw1": _bf16_np(W["bdw1"]), "bdmb": _bf16_np(W["bdmb"]),
        "jkw": _bf16_np(W["jkw"]), "clsw": _bf16_np(W["clsw"]),
        "clsbrow": _bf16_np(W["clsbrow"]), "extras": extras,
    }
    in_maps = []
    for k in range(NCORES):
        n0 = k * NSH
        xsh = np.zeros((P, NSHP), np.float32)
        xsh[:, :NSH] = x[n0 + perms[k]].T
        m = dict(common)
        m["xt"] = _bf16_np(xsh)
        m["st_idx"] = np.ascontiguousarray(idxs[k])
        in_maps.append(m)
    return cfg, in_maps, perms


def _ensure_ntff_hook():
    import contextlib
    import ctypes
    import sys
    import types

    try:
        from antenv.axon_hooks import get_axon_ntff_profile_hook  # noqa: F401
        return
    except ImportError:
        pass

    so_path = "/opt/axon/libaxon_pjrt.so"
    lib = ctypes.CDLL(so_path)
    if not hasattr(lib, "axon_start_nrt_profile"):
        return
    lib.axon_start_nrt_profile.argtypes = [
        ctypes.POINTER(ctypes.c_int64), ctypes.c_size_t]
    lib.axon_start_nrt_profile.restype = ctypes.c_int64
    lib.axon_stop_nrt_profile.argtypes = [ctypes.c_char_p]
    lib.axon_stop_nrt_profile.restype = ctypes.c_int64

    @contextlib.contextmanager
    def _hook(output_dir, device_ids):
        import jax
        jax.devices()
        if device_ids:
            ids = (ctypes.c_int64 * len(device_ids))(*device_ids)
            rc = lib.axon_start_nrt_profile(ids, len(device_ids))
        else:
            rc = lib.axon_start_nrt_profile(None, 0)
        if rc != 0:
            raise RuntimeError(f"axon_start_nrt_profile rc={rc}")
        try:
            yield
        finally:
            n = lib.axon_stop_nrt_profile(str(output_dir).encode())
            if n < 0:
                raise RuntimeError(f"axon_stop_nrt_profile rc={n}")
            print(f"profile: {n} file(s) written to {output_dir}")

    holder = {"h": _hook}
    mod = types.ModuleType("antenv.axon_hooks")
    mod.set_axon_ntff_profile_hook = lambda h: holder.__setitem__("h", h)
    mod.get_axon_ntff_profile_hook = lambda: holder.get("h")
    import antenv
    antenv.axon_hooks = mod
    sys.modules["antenv.axon_hooks"] = mod


def kernel(**inputs):
    global LAST_EXEC_NS, LAST_SCOPES
    from concourse.bass_utils import run_bass_kernel_spmd

    cfg, in_maps, perms = _prepare(inputs)
    key = tuple(sorted((k, v) for k, v in cfg.items()))
    if key not in _CACHE:
        _CACHE[key] = _build(cfg)
    nc = _CACHE[key]
    trace = bool(os.environ.get("KERNEL_TRACE"))
    kw = {}
    if trace:
        import tempfile
        try:
            _ensure_ntff_hook()
        except Exception:
            pass
        kw = dict(trace=True, tmpdir=tempfile.mkdtemp(prefix="k2trace_"))
    res = run_bass_kernel_spmd(nc, in_maps, core_ids=list(range(NCORES)), **kw)
    LAST_EXEC_NS = res.exec_time_ns
    LAST_SCOPES = res.per_core_scope_times
    NSH = cfg["NSH"]
    out = np.empty((cfg["N"], 40), np.float32)
    for k in range(NCORES):
        out[k * NSH + perms[k]] = res.results[k]["out"][:NSH]
    return out.astype(np.float32)
